# revision 33
# baseline (speedup 1.0000x reference)
"""DN4 retrieval-kNN layer as a Trainium2 Bass/Tile kernel (fp8 DoubleRow,
software-pipelined).

Reference computation (shapes hardcoded from the problem spec):
  query_feat  [t=4, wq=75, c=640, 10, 10]  -> q normalized over hw axis (per (wq, c))
  support_feat[t=4, ws=25, c=640, 10, 10]  -> s normalized over c axis (per (way, y))
  relation[t, wq, way, x, y] = sum_c qn[t, wq, x, c] * sn[t, way, c, y]   (x=100, y=500)
  score[t, wq, way] = sum_x sum(top3_y(relation))

Sharding: 8 cores = 4 episodes (t) x 2 query-halves. Core 2t handles queries
[0:38), core 2t+1 handles queries [37:75) (38 rows each; query 37 is computed
twice and deduplicated on the host). No cross-device communication.

Device kernel (per core), three stages connected as a pipeline:
  NORMS: support: ACT squares -> ones-matmul partition reduction (PE) ->
         ACT Abs_reciprocal_sqrt(x/256) = 16/||s|| -> ones outer-product
         broadcast (PE) -> ACT copy. query: ACT squares -> DVE strided
         segment reduce -> ACT Abs_reciprocal_sqrt -> 16/||q||.
  CASTS: gpsimd multiplies raw bf16 inputs by the 16/norm factors, writing
         fp8e4 tiles laid out for DoubleRow matmuls (k-chunk pairs).
  MAIN:  30 groups of 128 flattened (query, x) rows; per (group, way) the
         640-deep contraction runs as 2 DoubleRow fp8 matmuls (K=256) + 1
         plain fp8 matmul (K=128) into a PSUM bank; DVE max8 gives top-8 per
         row, a strided reduce sums top-3, and a segment-matrix matmul folds
         rows into per-query scores (the 1/256 fp8 scale is folded into the
         segment matrix).

The timed build unrolls two kernel instances per hardware-loop body and
ping-pongs two fixed buffer sets so that iteration i's casts and norms
overlap iteration i-1's matmul/top-k stream (the DVE queue is in-order, so
the q-norm reduces are also interleaved into the max8 stream to keep PSUM
slots recycling).
"""

import sys
import numpy as np

sys.path.insert(0, "/opt/trn_rl_repo")

T, WQ, C, HW = 4, 75, 640, 100
WAY, SHOT = 5, 5
NS = WAY * SHOT          # 25 support images per episode
Y = SHOT * HW            # 500 support descriptors per way
YALL = WAY * Y           # 2500
QPC = 38                 # queries per core (overlapping halves of 75)
KC = C // 128            # 5 contraction chunks of 128
NCORES = 8
NK = 3                   # top-k
ROWS = QPC * HW          # 3800 flattened (query, x) relation rows per core
GROUPS = (ROWS + 127) // 128   # 30 row-groups of <=128
BK = 512                 # PSUM bank stride in fp32 elements
PADY = 2512              # fp8 support row padded so the k-pair stride is %16
PADQ = 3808              # fp8 query row padded so the k-pair stride is %16
SCALE = 16.0             # fp8 pre-scale; 1/SCALE^2 folded into seg matrix
UNROLL = 6               # kernel instances per timed-loop body

_PROGRAM = None


def _build_program(phases=3, loop_reps=0, loop_scope="compute"):
    import concourse.tile as tile
    from concourse import bacc, mybir
    from contextlib import ExitStack

    fp32 = mybir.dt.float32
    bf16 = mybir.dt.bfloat16
    fp8 = mybir.dt.float8e4
    AF = mybir.ActivationFunctionType
    AX = mybir.AxisListType
    PM = mybir.MatmulPerfMode

    nc = bacc.Bacc("TRN2", target_bir_lowering=False, debug=False)
    q_in = nc.declare_dram_parameter("q_in", [C, QPC, HW], bf16, isOutput=False)
    s_in = nc.declare_dram_parameter("s_in", [C, NS, HW], bf16, isOutput=False)
    seg_in = nc.declare_dram_parameter("seg_in", [128, GROUPS, QPC], fp32, isOutput=False)
    score_out = nc.declare_dram_parameter("score_out", [QPC, WAY], fp32, isOutput=True)

    with ExitStack() as ctx:
        tc = ctx.enter_context(tile.TileContext(nc))
        const = ctx.enter_context(tc.tile_pool(name="const", bufs=1))
        sbig = ctx.enter_context(tc.tile_pool(name="sbig", bufs=1))
        ssq_pool = ctx.enter_context(tc.tile_pool(name="ssqp", bufs=5))
        qsq_pool = ctx.enter_context(tc.tile_pool(name="qsqp", bufs=2))
        small = ctx.enter_context(tc.tile_pool(name="small", bufs=2))
        t8p = ctx.enter_context(tc.tile_pool(name="t8p", bufs=8))
        psA = ctx.enter_context(tc.tile_pool(name="psA", bufs=7, space="PSUM"))
        psB = ctx.enter_context(tc.tile_pool(name="psB", bufs=1, space="PSUM"))

        # Constants
        ones_k = const.tile([128, 1], bf16, name="ones_k")
        nc.vector.memset(ones_k[:], 1.0)
        ones_m = const.tile([1, 128], bf16, name="ones_m")
        nc.vector.memset(ones_m[:], 1.0)

        seg = sbig.tile([128, GROUPS, QPC], fp32, name="seg")
        nc.sync.dma_start(out=seg[:], in_=seg_in[:])

        # ------------- loads (bf16 staging for normalization) -------------
        s_bf = []
        q_bf = []
        for kc in range(KC):
            sbk = sbig.tile([128, WAY, Y], bf16, name=f"sbf{kc}")
            s_bf.append(sbk)
            nc.sync.dma_start(
                out=sbk[:].rearrange("c w (s x) -> c (w s) x", x=HW),
                in_=s_in[kc * 128:(kc + 1) * 128],
            )
            qbk = sbig.tile([128, QPC, HW], bf16, name=f"qbf{kc}")
            q_bf.append(qbk)
            nc.sync.dma_start(out=qbk[:], in_=q_in[kc * 128:(kc + 1) * 128])

        # Fixed buffer sets (ping-pong across the unrolled body halves).
        nsets = 2 if loop_reps else 1

        class BufSet:
            pass

        sets = []
        for u in range(nsets):
            b = BufSet()
            b.sp = [sbig.tile([128, 2, PADY], fp8, name=f"sp{p}_{u}")
                    for p in range(2)]
            b.s5 = sbig.tile([128, YALL], fp8, name=f"s5_{u}")
            b.qp = [sbig.tile([128, 2, PADQ], fp8, name=f"qp{p}_{u}")
                    for p in range(2)]
            b.q5 = sbig.tile([128, ROWS], fp8, name=f"q5_{u}")
            b.rs_sb = sbig.tile([128, WAY, Y], bf16, name=f"rs_{u}")
            b.rq = sbig.tile([128, KC, QPC], fp32, name=f"rq_{u}")
            b.t3all = sbig.tile([128, GROUPS, WAY], fp32, name=f"t3_{u}")
            sets.append(b)

        def s_dst(b, kc):
            if kc < 4:
                return b.sp[kc // 2][:, kc % 2, 0:YALL]
            return b.s5[:]

        def q_dst(b, kc):
            if kc < 4:
                return b.qp[kc // 2][:, kc % 2, 0:ROWS]
            return b.q5[:]

        # ---------------- stage emitters ----------------
        QH = QPC // 2  # 19

        def emit_qnorm_half(b, kc, h):
            """ACT square + DVE segment-reduce + ACT rsqrt for half a k-chunk."""
            q0 = h * QH
            nq = QPC - q0 if h == 1 else QH
            sqq = qsq_pool.tile([128, QH * HW], bf16, name="sqq",
                                padded_shape=[128, (QPC - QH) * HW])
            nc.scalar.activation(
                sqq[:, 0:nq * HW],
                q_bf[kc][:, q0:q0 + nq].rearrange("c q x -> c (q x)"),
                AF.Square,
            )
            ssq = small.tile([128, QH], fp32, name="ssq",
                             padded_shape=[128, QPC - QH])
            nc.vector.reduce_sum(
                ssq[:, 0:nq],
                sqq[:, 0:nq * HW].rearrange("c (q x) -> c q x", x=HW),
                axis=AX.X,
            )
            # 1/sqrt(ss/256) = 16/||q||
            nc.scalar.activation(b.rq[:, kc, q0:q0 + nq], ssq[:, 0:nq],
                                 AF.Abs_reciprocal_sqrt,
                                 scale=1.0 / (SCALE * SCALE))

        def emit_qnorm_chunk(b, kc):
            emit_qnorm_half(b, kc, 0)
            emit_qnorm_half(b, kc, 1)

        def emit_s_squares(b):
            s_sq = []
            for kc in range(KC):
                sq = ssq_pool.tile([128, YALL], bf16, name="ssq_s")
                nc.scalar.activation(
                    sq[:], s_bf[kc][:].rearrange("c w y -> c (w y)"), AF.Square
                )
                s_sq.append(sq)
            return s_sq

        def emit_snorm_way(b, s_sq, s_rs, w):
            if True:
                ss = psB.tile([1, BK], fp32, name="ss", tag="aux")
                for kc in range(KC):
                    nc.tensor.matmul(
                        ss[:, 0:Y],
                        lhsT=ones_k[:],
                        rhs=s_sq[kc][:, w * Y:(w + 1) * Y],
                        start=(kc == 0),
                        stop=(kc == KC - 1),
                    )
                # 1/sqrt(ss/256) = 16/||s||
                nc.scalar.activation(
                    s_rs[:, w * Y:(w + 1) * Y], ss[:, 0:Y],
                    AF.Abs_reciprocal_sqrt, scale=1.0 / (SCALE * SCALE),
                )
                rb = psB.tile([128, BK], fp32, name="rb", tag="aux")
                nc.tensor.matmul(
                    rb[:, 0:Y],
                    lhsT=ones_m[:],
                    rhs=s_rs[:, w * Y:(w + 1) * Y],
                    start=True,
                    stop=True,
                )
                nc.scalar.copy(b.rs_sb[:, w], rb[:, 0:Y])

        def emit_snorm(b):
            s_sq = emit_s_squares(b)
            s_rs = small.tile([1, YALL], bf16, name="s_rs")
            for w in range(WAY):
                emit_snorm_way(b, s_sq, s_rs, w)

        def emit_casts(b):
            rs_flat = b.rs_sb[:].rearrange("c w y -> c (w y)")
            for kc in range(KC):
                nc.gpsimd.tensor_mul(
                    s_dst(b, kc),
                    s_bf[kc][:].rearrange("c w y -> c (w y)"),
                    rs_flat,
                )
            for kc in range(KC):
                nc.gpsimd.tensor_mul(
                    q_dst(b, kc).rearrange("c (q x) -> c q x", x=HW),
                    q_bf[kc][:],
                    b.rq[:, kc].unsqueeze(2).broadcast_to([128, QPC, HW]),
                )

        def emit_main(b, interleave=None):
            """Relation matmuls + top-3 for buffer set b.

            interleave: optional {group_index: callback} run between groups
            (used to spread the next iteration's q-norm DVE work through the
            max8 stream).
            """
            for g in range(GROUPS):
                if interleave and g in interleave:
                    interleave[g]()
                m = min(128, ROWS - g * 128)
                t8q = t8p.tile([128, WAY * 8], fp32, name="t8q")
                rels = [psA.tile([128, Y], fp32, name="rel", tag="rel")
                        for _ in range(WAY)]
                for p in range(2):
                    lhsT = b.qp[p][:, :, g * 128:g * 128 + m]
                    for w in range(WAY):
                        nc.tensor.matmul(
                            rels[w][0:m],
                            lhsT=lhsT,
                            rhs=b.sp[p][:, :, w * Y:(w + 1) * Y],
                            start=(p == 0),
                            stop=False,
                            perf_mode=PM.DoubleRow,
                            skip_group_check=True,
                        )
                lhsT5 = b.q5[:, g * 128:g * 128 + m]
                for w in range(WAY):
                    nc.tensor.matmul(
                        rels[w][0:m],
                        lhsT=lhsT5,
                        rhs=b.s5[:, w * Y:(w + 1) * Y],
                        start=False,
                        stop=True,
                        skip_group_check=True,
                    )
                    nc.vector.max(t8q[0:m, w * 8:(w + 1) * 8], rels[w][0:m])
                nc.vector.reduce_sum(
                    b.t3all[0:m, g],
                    t8q[:].rearrange("p (w k) -> p w k", k=8)[0:m, :, 0:NK],
                    axis=AX.X,
                )
            score_ps = psA.tile([QPC, WAY], fp32, name="score_ps", tag="rel")
            for g in range(GROUPS):
                m = min(128, ROWS - g * 128)
                nc.tensor.matmul(
                    score_ps[:],
                    lhsT=seg[0:m, g],
                    rhs=b.t3all[0:m, g],
                    start=(g == 0),
                    stop=(g == GROUPS - 1),
                )
            score_sb = small.tile([QPC, WAY], fp32, name="score_sb")
            nc.vector.tensor_copy(score_sb[:], score_ps[:])
            return score_sb

        # ---------------- program ----------------
        if not loop_reps:
            # single-shot: natural order
            b = sets[0]
            emit_snorm(b)
            for kc in range(KC):
                emit_qnorm_chunk(b, kc)
            emit_casts(b)
            if phases >= 3:
                score_sb = emit_main(b)
            else:
                score_sb = small.tile([QPC, WAY], fp32, name="score_sb")
                nc.vector.tensor_copy(score_sb[:], s_bf[0][0:QPC, 0, 0:WAY])
            nc.sync.dma_start(out=score_out[:], in_=score_sb[:])
        else:
            # timed: software-pipelined, UNROLL kernel instances per body.
            # main(set u) reads fp8 tiles written in the previous body half;
            # casts/norms for the next half overlap it.
            score_sb = None
            with tc.For_i(0, loop_reps, 1):
                for u in range(UNROLL):
                    b_cur = sets[u % 2]
                    b_nxt = sets[(u + 1) % 2]
                    if phases >= 3:
                        s_sq_nxt = emit_s_squares(b_nxt)
                        s_rs_nxt = small.tile([1, YALL], bf16, name="s_rs")
                        il = {8 + 2 * i: (lambda b=b_nxt, i=i:
                                          emit_qnorm_half(b, i // 2, i % 2))
                              for i in range(2 * KC)}
                        for w in range(WAY):
                            il[5 + 2 * w] = (
                                lambda b=b_nxt, w=w:
                                emit_snorm_way(b, s_sq_nxt, s_rs_nxt, w))
                        score_sb = emit_main(b_cur, interleave=il)
                        emit_casts(b_nxt)
                    else:
                        emit_snorm(b_nxt)
                        for kc in range(KC):
                            emit_qnorm_chunk(b_nxt, kc)
                        emit_casts(b_nxt)
            if score_sb is None:
                score_sb = small.tile([QPC, WAY], fp32, name="score_sb")
                nc.vector.tensor_copy(score_sb[:], s_bf[0][0:QPC, 0, 0:WAY])
            nc.sync.dma_start(out=score_out[:], in_=score_sb[:])

    nc.compile()
    return nc


def _get_program():
    global _PROGRAM
    if _PROGRAM is None:
        _PROGRAM = _build_program()
    return _PROGRAM


def _seg_matrix():
    seg = np.zeros((128, GROUPS, QPC), dtype=np.float32)
    inv = 1.0 / (SCALE * SCALE)
    for r in range(ROWS):
        seg[r % 128, r // 128, r // HW] = inv
    return seg


def _make_in_maps(qf, sf):
    import ml_dtypes
    bf = ml_dtypes.bfloat16
    seg = _seg_matrix()
    in_maps = []
    for core in range(NCORES):
        t = core // 2
        q0 = 0 if core % 2 == 0 else WQ - QPC  # 0 or 37
        in_maps.append({
            "q_in": np.ascontiguousarray(
                qf[t, q0:q0 + QPC].transpose(1, 0, 2).astype(bf)),
            "s_in": np.ascontiguousarray(
                sf[t].transpose(1, 0, 2).astype(bf)),
            "seg_in": seg,
        })
    return in_maps


def kernel(query_feat, support_feat, way_num, shot_num, query_num, **_):
    from concourse.bass_utils import run_bass_kernel_spmd

    qf = np.asarray(query_feat, dtype=np.float32).reshape(T, WQ, C, HW)
    sf = np.asarray(support_feat, dtype=np.float32).reshape(T, NS, C, HW)
    assert int(way_num) == WAY and int(shot_num) == SHOT

    in_maps = _make_in_maps(qf, sf)
    res = run_bass_kernel_spmd(_get_program(), in_maps, list(range(NCORES))).results

    out = np.empty((T, WQ, WAY), dtype=np.float32)
    for t in range(T):
        lo = res[2 * t]["score_out"]
        hi = res[2 * t + 1]["score_out"]
        out[t, :QPC] = lo
        out[t, QPC:] = hi[QPC - (WQ - QPC):]  # drop the overlapping query row
    return out


# revision 34
# speedup vs baseline: 1.0021x; 1.0021x over previous
"""DN4 retrieval-kNN layer as a Trainium2 Bass/Tile kernel (fp8 DoubleRow,
software-pipelined).

Reference computation (shapes hardcoded from the problem spec):
  query_feat  [t=4, wq=75, c=640, 10, 10]  -> q normalized over hw axis (per (wq, c))
  support_feat[t=4, ws=25, c=640, 10, 10]  -> s normalized over c axis (per (way, y))
  relation[t, wq, way, x, y] = sum_c qn[t, wq, x, c] * sn[t, way, c, y]   (x=100, y=500)
  score[t, wq, way] = sum_x sum(top3_y(relation))

Sharding: 8 cores = 4 episodes (t) x 2 query-halves. Core 2t handles queries
[0:38), core 2t+1 handles queries [37:75) (38 rows each; query 37 is computed
twice and deduplicated on the host). No cross-device communication.

Device kernel (per core), three stages connected as a pipeline:
  NORMS: support: ACT squares -> ones-matmul partition reduction (PE) ->
         ACT Abs_reciprocal_sqrt(x/256) = 16/||s|| -> ones outer-product
         broadcast (PE) -> ACT copy. query: ACT squares -> DVE strided
         segment reduce -> ACT Abs_reciprocal_sqrt -> 16/||q||.
  CASTS: gpsimd multiplies raw bf16 inputs by the 16/norm factors, writing
         fp8e4 tiles laid out for DoubleRow matmuls (k-chunk pairs).
  MAIN:  30 groups of 128 flattened (query, x) rows; per (group, way) the
         640-deep contraction runs as 2 DoubleRow fp8 matmuls (K=256) + 1
         plain fp8 matmul (K=128) into a PSUM bank; DVE max8 gives top-8 per
         row, a strided reduce sums top-3, and a segment-matrix matmul folds
         rows into per-query scores (the 1/256 fp8 scale is folded into the
         segment matrix).

The timed build unrolls two kernel instances per hardware-loop body and
ping-pongs two fixed buffer sets so that iteration i's casts and norms
overlap iteration i-1's matmul/top-k stream (the DVE queue is in-order, so
the q-norm reduces are also interleaved into the max8 stream to keep PSUM
slots recycling).
"""

import sys
import numpy as np

sys.path.insert(0, "/opt/trn_rl_repo")

T, WQ, C, HW = 4, 75, 640, 100
WAY, SHOT = 5, 5
NS = WAY * SHOT          # 25 support images per episode
Y = SHOT * HW            # 500 support descriptors per way
YALL = WAY * Y           # 2500
QPC = 38                 # queries per core (overlapping halves of 75)
KC = C // 128            # 5 contraction chunks of 128
NCORES = 8
NK = 3                   # top-k
ROWS = QPC * HW          # 3800 flattened (query, x) relation rows per core
GROUPS = (ROWS + 127) // 128   # 30 row-groups of <=128
BK = 512                 # PSUM bank stride in fp32 elements
PADY = 2512              # fp8 support row padded so the k-pair stride is %16
PADQ = 3808              # fp8 query row padded so the k-pair stride is %16
SCALE = 16.0             # fp8 pre-scale; 1/SCALE^2 folded into seg matrix
UNROLL = 6               # kernel instances per timed-loop body

_PROGRAM = None


def _build_program(phases=3, loop_reps=0, loop_scope="compute"):
    import concourse.tile as tile
    from concourse import bacc, mybir
    from contextlib import ExitStack

    fp32 = mybir.dt.float32
    bf16 = mybir.dt.bfloat16
    fp8 = mybir.dt.float8e4
    AF = mybir.ActivationFunctionType
    AX = mybir.AxisListType
    PM = mybir.MatmulPerfMode

    nc = bacc.Bacc("TRN2", target_bir_lowering=False, debug=False)
    q_in = nc.declare_dram_parameter("q_in", [C, QPC, HW], bf16, isOutput=False)
    s_in = nc.declare_dram_parameter("s_in", [C, NS, HW], bf16, isOutput=False)
    seg_in = nc.declare_dram_parameter("seg_in", [128, GROUPS, QPC], fp32, isOutput=False)
    score_out = nc.declare_dram_parameter("score_out", [QPC, WAY], fp32, isOutput=True)

    with ExitStack() as ctx:
        tc = ctx.enter_context(tile.TileContext(nc))
        const = ctx.enter_context(tc.tile_pool(name="const", bufs=1))
        sbig = ctx.enter_context(tc.tile_pool(name="sbig", bufs=1))
        ssq_pool = ctx.enter_context(tc.tile_pool(name="ssqp", bufs=5))
        qsq_pool = ctx.enter_context(tc.tile_pool(name="qsqp", bufs=2))
        small = ctx.enter_context(tc.tile_pool(name="small", bufs=2))
        t8p = ctx.enter_context(tc.tile_pool(name="t8p", bufs=8))
        psA = ctx.enter_context(tc.tile_pool(name="psA", bufs=7, space="PSUM"))
        psB = ctx.enter_context(tc.tile_pool(name="psB", bufs=1, space="PSUM"))

        # Constants
        ones_k = const.tile([128, 1], bf16, name="ones_k")
        nc.vector.memset(ones_k[:], 1.0)
        ones_m = const.tile([1, 128], bf16, name="ones_m")
        nc.vector.memset(ones_m[:], 1.0)

        seg = sbig.tile([128, GROUPS, QPC], fp32, name="seg")
        nc.sync.dma_start(out=seg[:], in_=seg_in[:])

        # ------------- loads (bf16 staging for normalization) -------------
        s_bf = []
        q_bf = []
        for kc in range(KC):
            sbk = sbig.tile([128, WAY, Y], bf16, name=f"sbf{kc}")
            s_bf.append(sbk)
            nc.sync.dma_start(
                out=sbk[:].rearrange("c w (s x) -> c (w s) x", x=HW),
                in_=s_in[kc * 128:(kc + 1) * 128],
            )
            qbk = sbig.tile([128, QPC, HW], bf16, name=f"qbf{kc}")
            q_bf.append(qbk)
            nc.sync.dma_start(out=qbk[:], in_=q_in[kc * 128:(kc + 1) * 128])

        # Fixed buffer sets (ping-pong across the unrolled body halves).
        nsets = 2 if loop_reps else 1

        class BufSet:
            pass

        sets = []
        for u in range(nsets):
            b = BufSet()
            b.sp = [sbig.tile([128, 2, PADY], fp8, name=f"sp{p}_{u}")
                    for p in range(2)]
            b.s5 = sbig.tile([128, YALL], fp8, name=f"s5_{u}")
            b.qp = [sbig.tile([128, 2, PADQ], fp8, name=f"qp{p}_{u}")
                    for p in range(2)]
            b.q5 = sbig.tile([128, ROWS], fp8, name=f"q5_{u}")
            b.rs_sb = sbig.tile([128, WAY, Y], bf16, name=f"rs_{u}")
            b.rq = sbig.tile([128, KC, QPC], fp32, name=f"rq_{u}")
            b.t3all = sbig.tile([128, GROUPS, WAY], fp32, name=f"t3_{u}")
            sets.append(b)

        def s_dst(b, kc):
            if kc < 4:
                return b.sp[kc // 2][:, kc % 2, 0:YALL]
            return b.s5[:]

        def q_dst(b, kc):
            if kc < 4:
                return b.qp[kc // 2][:, kc % 2, 0:ROWS]
            return b.q5[:]

        # ---------------- stage emitters ----------------
        QH = QPC // 2  # 19

        def emit_qnorm_half(b, kc, h):
            """ACT square + DVE segment-reduce + ACT rsqrt for half a k-chunk."""
            q0 = h * QH
            nq = QPC - q0 if h == 1 else QH
            sqq = qsq_pool.tile([128, QH * HW], bf16, name="sqq",
                                padded_shape=[128, (QPC - QH) * HW])
            nc.scalar.activation(
                sqq[:, 0:nq * HW],
                q_bf[kc][:, q0:q0 + nq].rearrange("c q x -> c (q x)"),
                AF.Square,
            )
            ssq = small.tile([128, QH], fp32, name="ssq",
                             padded_shape=[128, QPC - QH])
            nc.vector.reduce_sum(
                ssq[:, 0:nq],
                sqq[:, 0:nq * HW].rearrange("c (q x) -> c q x", x=HW),
                axis=AX.X,
            )
            # 1/sqrt(ss/256) = 16/||q||
            nc.scalar.activation(b.rq[:, kc, q0:q0 + nq], ssq[:, 0:nq],
                                 AF.Abs_reciprocal_sqrt,
                                 scale=1.0 / (SCALE * SCALE))

        def emit_qnorm_chunk(b, kc):
            emit_qnorm_half(b, kc, 0)
            emit_qnorm_half(b, kc, 1)

        def emit_snorm(b):
            s_sq = []
            for kc in range(KC):
                sq = ssq_pool.tile([128, YALL], bf16, name="ssq_s")
                nc.scalar.activation(
                    sq[:], s_bf[kc][:].rearrange("c w y -> c (w y)"), AF.Square
                )
                s_sq.append(sq)
            s_rs = small.tile([1, YALL], bf16, name="s_rs")
            for w in range(WAY):
                ss = psB.tile([1, BK], fp32, name="ss", tag="aux")
                for kc in range(KC):
                    nc.tensor.matmul(
                        ss[:, 0:Y],
                        lhsT=ones_k[:],
                        rhs=s_sq[kc][:, w * Y:(w + 1) * Y],
                        start=(kc == 0),
                        stop=(kc == KC - 1),
                    )
                # 1/sqrt(ss/256) = 16/||s||
                nc.scalar.activation(
                    s_rs[:, w * Y:(w + 1) * Y], ss[:, 0:Y],
                    AF.Abs_reciprocal_sqrt, scale=1.0 / (SCALE * SCALE),
                )
                rb = psB.tile([128, BK], fp32, name="rb", tag="aux")
                nc.tensor.matmul(
                    rb[:, 0:Y],
                    lhsT=ones_m[:],
                    rhs=s_rs[:, w * Y:(w + 1) * Y],
                    start=True,
                    stop=True,
                )
                nc.scalar.copy(b.rs_sb[:, w], rb[:, 0:Y])

        def emit_casts(b):
            rs_flat = b.rs_sb[:].rearrange("c w y -> c (w y)")
            for kc in range(KC):
                nc.gpsimd.tensor_mul(
                    s_dst(b, kc),
                    s_bf[kc][:].rearrange("c w y -> c (w y)"),
                    rs_flat,
                )
            for kc in range(KC):
                nc.gpsimd.tensor_mul(
                    q_dst(b, kc).rearrange("c (q x) -> c q x", x=HW),
                    q_bf[kc][:],
                    b.rq[:, kc].unsqueeze(2).broadcast_to([128, QPC, HW]),
                )

        def emit_main(b, interleave=None):
            """Relation matmuls + top-3 for buffer set b.

            interleave: optional {group_index: callback} run between groups
            (used to spread the next iteration's q-norm DVE work through the
            max8 stream).
            """
            for g in range(GROUPS):
                if interleave and g in interleave:
                    interleave[g]()
                m = min(128, ROWS - g * 128)
                t8q = t8p.tile([128, WAY * 8], fp32, name="t8q")
                rels = [psA.tile([128, Y], fp32, name="rel", tag="rel")
                        for _ in range(WAY)]
                for p in range(2):
                    lhsT = b.qp[p][:, :, g * 128:g * 128 + m]
                    for w in range(WAY):
                        nc.tensor.matmul(
                            rels[w][0:m],
                            lhsT=lhsT,
                            rhs=b.sp[p][:, :, w * Y:(w + 1) * Y],
                            start=(p == 0),
                            stop=False,
                            perf_mode=PM.DoubleRow,
                            skip_group_check=True,
                        )
                lhsT5 = b.q5[:, g * 128:g * 128 + m]
                for w in range(WAY):
                    nc.tensor.matmul(
                        rels[w][0:m],
                        lhsT=lhsT5,
                        rhs=b.s5[:, w * Y:(w + 1) * Y],
                        start=False,
                        stop=True,
                        skip_group_check=True,
                    )
                    nc.vector.max(t8q[0:m, w * 8:(w + 1) * 8], rels[w][0:m])
                nc.vector.reduce_sum(
                    b.t3all[0:m, g],
                    t8q[:].rearrange("p (w k) -> p w k", k=8)[0:m, :, 0:NK],
                    axis=AX.X,
                )
            score_ps = psA.tile([QPC, WAY], fp32, name="score_ps", tag="rel")
            for g in range(GROUPS):
                m = min(128, ROWS - g * 128)
                nc.tensor.matmul(
                    score_ps[:],
                    lhsT=seg[0:m, g],
                    rhs=b.t3all[0:m, g],
                    start=(g == 0),
                    stop=(g == GROUPS - 1),
                )
            score_sb = small.tile([QPC, WAY], fp32, name="score_sb")
            nc.vector.tensor_copy(score_sb[:], score_ps[:])
            return score_sb

        # ---------------- program ----------------
        if not loop_reps:
            # single-shot: natural order
            b = sets[0]
            emit_snorm(b)
            for kc in range(KC):
                emit_qnorm_chunk(b, kc)
            emit_casts(b)
            if phases >= 3:
                score_sb = emit_main(b)
            else:
                score_sb = small.tile([QPC, WAY], fp32, name="score_sb")
                nc.vector.tensor_copy(score_sb[:], s_bf[0][0:QPC, 0, 0:WAY])
            nc.sync.dma_start(out=score_out[:], in_=score_sb[:])
        else:
            # timed: software-pipelined, UNROLL kernel instances per body.
            # main(set u) reads fp8 tiles written in the previous body half;
            # casts/norms for the next half overlap it.
            score_sb = None
            with tc.For_i(0, loop_reps, 1):
                for u in range(UNROLL):
                    b_cur = sets[u % 2]
                    b_nxt = sets[(u + 1) % 2]
                    if phases >= 3:
                        il = {8 + 2 * i: (lambda b=b_nxt, i=i:
                                          emit_qnorm_half(b, i // 2, i % 2))
                              for i in range(2 * KC)}
                        score_sb = emit_main(b_cur, interleave=il)
                        emit_snorm(b_nxt)
                        emit_casts(b_nxt)
                    else:
                        emit_snorm(b_nxt)
                        for kc in range(KC):
                            emit_qnorm_chunk(b_nxt, kc)
                        emit_casts(b_nxt)
            if score_sb is None:
                score_sb = small.tile([QPC, WAY], fp32, name="score_sb")
                nc.vector.tensor_copy(score_sb[:], s_bf[0][0:QPC, 0, 0:WAY])
            nc.sync.dma_start(out=score_out[:], in_=score_sb[:])

    nc.compile()
    return nc


def _get_program():
    global _PROGRAM
    if _PROGRAM is None:
        _PROGRAM = _build_program()
    return _PROGRAM


def _seg_matrix():
    seg = np.zeros((128, GROUPS, QPC), dtype=np.float32)
    inv = 1.0 / (SCALE * SCALE)
    for r in range(ROWS):
        seg[r % 128, r // 128, r // HW] = inv
    return seg


def _make_in_maps(qf, sf):
    import ml_dtypes
    bf = ml_dtypes.bfloat16
    seg = _seg_matrix()
    in_maps = []
    for core in range(NCORES):
        t = core // 2
        q0 = 0 if core % 2 == 0 else WQ - QPC  # 0 or 37
        in_maps.append({
            "q_in": np.ascontiguousarray(
                qf[t, q0:q0 + QPC].transpose(1, 0, 2).astype(bf)),
            "s_in": np.ascontiguousarray(
                sf[t].transpose(1, 0, 2).astype(bf)),
            "seg_in": seg,
        })
    return in_maps


def kernel(query_feat, support_feat, way_num, shot_num, query_num, **_):
    from concourse.bass_utils import run_bass_kernel_spmd

    qf = np.asarray(query_feat, dtype=np.float32).reshape(T, WQ, C, HW)
    sf = np.asarray(support_feat, dtype=np.float32).reshape(T, NS, C, HW)
    assert int(way_num) == WAY and int(shot_num) == SHOT

    in_maps = _make_in_maps(qf, sf)
    res = run_bass_kernel_spmd(_get_program(), in_maps, list(range(NCORES))).results

    out = np.empty((T, WQ, WAY), dtype=np.float32)
    for t in range(T):
        lo = res[2 * t]["score_out"]
        hi = res[2 * t + 1]["score_out"]
        out[t, :QPC] = lo
        out[t, QPC:] = hi[QPC - (WQ - QPC):]  # drop the overlapping query row
    return out


# revision 35
# speedup vs baseline: 1.0200x; 1.0179x over previous
"""DN4 retrieval-kNN layer as a Trainium2 Bass/Tile kernel (fp8 DoubleRow,
software-pipelined).

Reference computation (shapes hardcoded from the problem spec):
  query_feat  [t=4, wq=75, c=640, 10, 10]  -> q normalized over hw axis (per (wq, c))
  support_feat[t=4, ws=25, c=640, 10, 10]  -> s normalized over c axis (per (way, y))
  relation[t, wq, way, x, y] = sum_c qn[t, wq, x, c] * sn[t, way, c, y]   (x=100, y=500)
  score[t, wq, way] = sum_x sum(top3_y(relation))

Sharding: 8 cores = 4 episodes (t) x 2 query-halves. Core 2t handles queries
[0:38), core 2t+1 handles queries [37:75) (38 rows each; query 37 is computed
twice and deduplicated on the host). No cross-device communication.

Device kernel (per core), three stages connected as a pipeline:
  NORMS: support: ACT squares -> ones-matmul partition reduction (PE) ->
         ACT Abs_reciprocal_sqrt(x/256) = 16/||s|| -> ones outer-product
         broadcast (PE) -> ACT copy. query: ACT squares -> DVE strided
         segment reduce -> ACT Abs_reciprocal_sqrt -> 16/||q||.
  CASTS: gpsimd multiplies raw bf16 inputs by the 16/norm factors, writing
         fp8e4 tiles laid out for DoubleRow matmuls (k-chunk pairs).
  MAIN:  30 groups of 128 flattened (query, x) rows; per (group, way) the
         640-deep contraction runs as 2 DoubleRow fp8 matmuls (K=256) + 1
         plain fp8 matmul (K=128) into a PSUM bank; DVE max8 gives top-8 per
         row, a strided reduce sums top-3, and a segment-matrix matmul folds
         rows into per-query scores (the 1/256 fp8 scale is folded into the
         segment matrix).

The timed build unrolls two kernel instances per hardware-loop body and
ping-pongs two fixed buffer sets so that iteration i's casts and norms
overlap iteration i-1's matmul/top-k stream (the DVE queue is in-order, so
the q-norm reduces are also interleaved into the max8 stream to keep PSUM
slots recycling).
"""

import sys
import numpy as np

sys.path.insert(0, "/opt/trn_rl_repo")

T, WQ, C, HW = 4, 75, 640, 100
WAY, SHOT = 5, 5
NS = WAY * SHOT          # 25 support images per episode
Y = SHOT * HW            # 500 support descriptors per way
YALL = WAY * Y           # 2500
QPC = 38                 # queries per core (overlapping halves of 75)
KC = C // 128            # 5 contraction chunks of 128
NCORES = 8
NK = 3                   # top-k
ROWS = QPC * HW          # 3800 flattened (query, x) relation rows per core
GROUPS = (ROWS + 127) // 128   # 30 row-groups of <=128
BK = 512                 # PSUM bank stride in fp32 elements
PADY = 2512              # fp8 support row padded so the k-pair stride is %16
PADQ = 3808              # fp8 query row padded so the k-pair stride is %16
SCALE = 16.0             # fp8 pre-scale; 1/SCALE^2 folded into seg matrix
UNROLL = 6               # kernel instances per timed-loop body

_PROGRAM = None


def _build_program(phases=3, loop_reps=0, loop_scope="compute"):
    import concourse.tile as tile
    from concourse import bacc, mybir
    from contextlib import ExitStack

    fp32 = mybir.dt.float32
    bf16 = mybir.dt.bfloat16
    fp8 = mybir.dt.float8e4
    AF = mybir.ActivationFunctionType
    AX = mybir.AxisListType
    PM = mybir.MatmulPerfMode

    nc = bacc.Bacc("TRN2", target_bir_lowering=False, debug=False)
    q_in = nc.declare_dram_parameter("q_in", [C, QPC, HW], bf16, isOutput=False)
    s_in = nc.declare_dram_parameter("s_in", [C, NS, HW], bf16, isOutput=False)
    seg_in = nc.declare_dram_parameter("seg_in", [128, GROUPS, QPC], fp32, isOutput=False)
    score_out = nc.declare_dram_parameter("score_out", [QPC, WAY], fp32, isOutput=True)

    with ExitStack() as ctx:
        tc = ctx.enter_context(tile.TileContext(nc))
        const = ctx.enter_context(tc.tile_pool(name="const", bufs=1))
        sbig = ctx.enter_context(tc.tile_pool(name="sbig", bufs=1))
        ssq_pool = ctx.enter_context(tc.tile_pool(name="ssqp", bufs=5))
        qsq_pool = ctx.enter_context(tc.tile_pool(name="qsqp", bufs=2))
        small = ctx.enter_context(tc.tile_pool(name="small", bufs=2))
        t8p = ctx.enter_context(tc.tile_pool(name="t8p", bufs=8))
        psA = ctx.enter_context(tc.tile_pool(name="psA", bufs=7, space="PSUM"))
        psB = ctx.enter_context(tc.tile_pool(name="psB", bufs=1, space="PSUM"))

        # Constants
        ones_k = const.tile([128, 1], bf16, name="ones_k")
        nc.vector.memset(ones_k[:], 1.0)
        ones_m = const.tile([1, 128], bf16, name="ones_m")
        nc.vector.memset(ones_m[:], 1.0)

        seg = sbig.tile([128, GROUPS, QPC], fp32, name="seg")
        nc.sync.dma_start(out=seg[:], in_=seg_in[:])

        # ------------- loads (bf16 staging for normalization) -------------
        s_bf = []
        q_bf = []
        for kc in range(KC):
            sbk = sbig.tile([128, WAY, Y], bf16, name=f"sbf{kc}")
            s_bf.append(sbk)
            nc.sync.dma_start(
                out=sbk[:].rearrange("c w (s x) -> c (w s) x", x=HW),
                in_=s_in[kc * 128:(kc + 1) * 128],
            )
            qbk = sbig.tile([128, QPC, HW], bf16, name=f"qbf{kc}")
            q_bf.append(qbk)
            nc.sync.dma_start(out=qbk[:], in_=q_in[kc * 128:(kc + 1) * 128])

        # Fixed buffer sets (ping-pong across the unrolled body halves).
        nsets = 2 if loop_reps else 1

        class BufSet:
            pass

        sets = []
        for u in range(nsets):
            b = BufSet()
            b.sp = [sbig.tile([128, 2, PADY], fp8, name=f"sp{p}_{u}")
                    for p in range(3)]
            b.qp = [sbig.tile([128, 2, PADQ], fp8, name=f"qp{p}_{u}")
                    for p in range(3)]
            nc.vector.memset(b.sp[2][:, 1], 0.0)
            nc.vector.memset(b.qp[2][:, 1], 0.0)
            b.rs_sb = sbig.tile([128, WAY, Y], bf16, name=f"rs_{u}")
            b.rq = sbig.tile([128, KC, QPC], fp32, name=f"rq_{u}")
            b.t3all = sbig.tile([128, GROUPS, WAY], fp32, name=f"t3_{u}")
            sets.append(b)

        def s_dst(b, kc):
            return b.sp[kc // 2][:, kc % 2, 0:YALL]

        def q_dst(b, kc):
            return b.qp[kc // 2][:, kc % 2, 0:ROWS]

        # ---------------- stage emitters ----------------
        QH = QPC // 2  # 19

        def emit_qnorm_half(b, kc, h):
            """ACT square + DVE segment-reduce + ACT rsqrt for half a k-chunk."""
            q0 = h * QH
            nq = QPC - q0 if h == 1 else QH
            sqq = qsq_pool.tile([128, QH * HW], bf16, name="sqq",
                                padded_shape=[128, (QPC - QH) * HW])
            nc.scalar.activation(
                sqq[:, 0:nq * HW],
                q_bf[kc][:, q0:q0 + nq].rearrange("c q x -> c (q x)"),
                AF.Square,
            )
            ssq = small.tile([128, QH], fp32, name="ssq",
                             padded_shape=[128, QPC - QH])
            nc.vector.reduce_sum(
                ssq[:, 0:nq],
                sqq[:, 0:nq * HW].rearrange("c (q x) -> c q x", x=HW),
                axis=AX.X,
            )
            # 1/sqrt(ss/256) = 16/||q||
            nc.scalar.activation(b.rq[:, kc, q0:q0 + nq], ssq[:, 0:nq],
                                 AF.Abs_reciprocal_sqrt,
                                 scale=1.0 / (SCALE * SCALE))

        def emit_qnorm_chunk(b, kc):
            emit_qnorm_half(b, kc, 0)
            emit_qnorm_half(b, kc, 1)

        def emit_snorm(b):
            s_sq = []
            for kc in range(KC):
                sq = ssq_pool.tile([128, YALL], bf16, name="ssq_s")
                nc.scalar.activation(
                    sq[:], s_bf[kc][:].rearrange("c w y -> c (w y)"), AF.Square
                )
                s_sq.append(sq)
            s_rs = small.tile([1, YALL], bf16, name="s_rs")
            for w in range(WAY):
                ss = psB.tile([1, BK], fp32, name="ss", tag="aux")
                for kc in range(KC):
                    nc.tensor.matmul(
                        ss[:, 0:Y],
                        lhsT=ones_k[:],
                        rhs=s_sq[kc][:, w * Y:(w + 1) * Y],
                        start=(kc == 0),
                        stop=(kc == KC - 1),
                    )
                # 1/sqrt(ss/256) = 16/||s||
                nc.scalar.activation(
                    s_rs[:, w * Y:(w + 1) * Y], ss[:, 0:Y],
                    AF.Abs_reciprocal_sqrt, scale=1.0 / (SCALE * SCALE),
                )
                rb = psB.tile([128, BK], fp32, name="rb", tag="aux")
                nc.tensor.matmul(
                    rb[:, 0:Y],
                    lhsT=ones_m[:],
                    rhs=s_rs[:, w * Y:(w + 1) * Y],
                    start=True,
                    stop=True,
                )
                nc.scalar.copy(b.rs_sb[:, w], rb[:, 0:Y])

        def emit_casts(b):
            rs_flat = b.rs_sb[:].rearrange("c w y -> c (w y)")
            for kc in range(KC):
                nc.gpsimd.tensor_mul(
                    s_dst(b, kc),
                    s_bf[kc][:].rearrange("c w y -> c (w y)"),
                    rs_flat,
                )
            for kc in range(KC):
                nc.gpsimd.tensor_mul(
                    q_dst(b, kc).rearrange("c (q x) -> c q x", x=HW),
                    q_bf[kc][:],
                    b.rq[:, kc].unsqueeze(2).broadcast_to([128, QPC, HW]),
                )

        def emit_main(b, interleave=None):
            """Relation matmuls + top-3 for buffer set b.

            interleave: optional {group_index: callback} run between groups
            (used to spread the next iteration's q-norm DVE work through the
            max8 stream).
            """
            for g in range(GROUPS):
                if interleave and g in interleave:
                    interleave[g]()
                m = min(128, ROWS - g * 128)
                t8q = t8p.tile([128, WAY * 8], fp32, name="t8q")
                rels = [psA.tile([128, Y], fp32, name="rel", tag="rel")
                        for _ in range(WAY)]
                for p in range(3):
                    lhsT = b.qp[p][:, :, g * 128:g * 128 + m]
                    for w in range(WAY):
                        nc.tensor.matmul(
                            rels[w][0:m],
                            lhsT=lhsT,
                            rhs=b.sp[p][:, :, w * Y:(w + 1) * Y],
                            start=(p == 0),
                            stop=(p == 2),
                            perf_mode=PM.DoubleRow,
                            skip_group_check=True,
                        )
                for w in range(WAY):
                    nc.vector.max(t8q[0:m, w * 8:(w + 1) * 8], rels[w][0:m])
                nc.vector.reduce_sum(
                    b.t3all[0:m, g],
                    t8q[:].rearrange("p (w k) -> p w k", k=8)[0:m, :, 0:NK],
                    axis=AX.X,
                )
            score_ps = psA.tile([QPC, WAY], fp32, name="score_ps", tag="rel")
            for g in range(GROUPS):
                m = min(128, ROWS - g * 128)
                nc.tensor.matmul(
                    score_ps[:],
                    lhsT=seg[0:m, g],
                    rhs=b.t3all[0:m, g],
                    start=(g == 0),
                    stop=(g == GROUPS - 1),
                )
            score_sb = small.tile([QPC, WAY], fp32, name="score_sb")
            nc.vector.tensor_copy(score_sb[:], score_ps[:])
            return score_sb

        # ---------------- program ----------------
        if not loop_reps:
            # single-shot: natural order
            b = sets[0]
            emit_snorm(b)
            for kc in range(KC):
                emit_qnorm_chunk(b, kc)
            emit_casts(b)
            if phases >= 3:
                score_sb = emit_main(b)
            else:
                score_sb = small.tile([QPC, WAY], fp32, name="score_sb")
                nc.vector.tensor_copy(score_sb[:], s_bf[0][0:QPC, 0, 0:WAY])
            nc.sync.dma_start(out=score_out[:], in_=score_sb[:])
        else:
            # timed: software-pipelined, UNROLL kernel instances per body.
            # main(set u) reads fp8 tiles written in the previous body half;
            # casts/norms for the next half overlap it.
            score_sb = None
            with tc.For_i(0, loop_reps, 1):
                for u in range(UNROLL):
                    b_cur = sets[u % 2]
                    b_nxt = sets[(u + 1) % 2]
                    if phases >= 3:
                        il = {8 + 2 * i: (lambda b=b_nxt, i=i:
                                          emit_qnorm_half(b, i // 2, i % 2))
                              for i in range(2 * KC)}
                        score_sb = emit_main(b_cur, interleave=il)
                        emit_snorm(b_nxt)
                        emit_casts(b_nxt)
                    else:
                        emit_snorm(b_nxt)
                        for kc in range(KC):
                            emit_qnorm_chunk(b_nxt, kc)
                        emit_casts(b_nxt)
            if score_sb is None:
                score_sb = small.tile([QPC, WAY], fp32, name="score_sb")
                nc.vector.tensor_copy(score_sb[:], s_bf[0][0:QPC, 0, 0:WAY])
            nc.sync.dma_start(out=score_out[:], in_=score_sb[:])

    nc.compile()
    return nc


def _get_program():
    global _PROGRAM
    if _PROGRAM is None:
        _PROGRAM = _build_program()
    return _PROGRAM


def _seg_matrix():
    seg = np.zeros((128, GROUPS, QPC), dtype=np.float32)
    inv = 1.0 / (SCALE * SCALE)
    for r in range(ROWS):
        seg[r % 128, r // 128, r // HW] = inv
    return seg


def _make_in_maps(qf, sf):
    import ml_dtypes
    bf = ml_dtypes.bfloat16
    seg = _seg_matrix()
    in_maps = []
    for core in range(NCORES):
        t = core // 2
        q0 = 0 if core % 2 == 0 else WQ - QPC  # 0 or 37
        in_maps.append({
            "q_in": np.ascontiguousarray(
                qf[t, q0:q0 + QPC].transpose(1, 0, 2).astype(bf)),
            "s_in": np.ascontiguousarray(
                sf[t].transpose(1, 0, 2).astype(bf)),
            "seg_in": seg,
        })
    return in_maps


def kernel(query_feat, support_feat, way_num, shot_num, query_num, **_):
    from concourse.bass_utils import run_bass_kernel_spmd

    qf = np.asarray(query_feat, dtype=np.float32).reshape(T, WQ, C, HW)
    sf = np.asarray(support_feat, dtype=np.float32).reshape(T, NS, C, HW)
    assert int(way_num) == WAY and int(shot_num) == SHOT

    in_maps = _make_in_maps(qf, sf)
    res = run_bass_kernel_spmd(_get_program(), in_maps, list(range(NCORES))).results

    out = np.empty((T, WQ, WAY), dtype=np.float32)
    for t in range(T):
        lo = res[2 * t]["score_out"]
        hi = res[2 * t + 1]["score_out"]
        out[t, :QPC] = lo
        out[t, QPC:] = hi[QPC - (WQ - QPC):]  # drop the overlapping query row
    return out
